# revision 1
# baseline (speedup 1.0000x reference)
"""Poincare pairwise edge generator on 8 Trainium2 NeuronCores.

Math: for the Poincare-ball distance with c=1, the mobius-norm numerator
factors exactly:  num2 = A^2|x|^2 - 2AB<x,y> + B^2|y|^2 = sqdist * D with
sqdist = |x-y|^2 and D = 1 - 2<x,y> + |x|^2|y|^2, so
  z = sqrt(sqdist/D) = exp(0.5*(ln sqdist - ln D))
  dists = ln(1+z) - ln(1-z)           (= 2 artanh z)
  probs = sigmoid(-dists) = (1-z)/2   (exact simplification)

-2<x_i,x_j> comes from a K=256 float32r matmul; u=|x_j|^2 is added via a
broadcast tile on DVE, s=|x_i|^2 via the activation bias port. The whole
per-element chain is 5 ACT ops + 4 DVE ops, pipelined in raw bass with
one-phase skew so cross-engine semaphore latency overlaps.

Sharding: rows across 8 cores (1024 each); every core holds the full
[256, 8192] transposed embeddings. Diagonal fixed on host at unshard
(probs diagonal is exactly 0; reference dists diagonal is fp32
cancellation noise of order 1e-4 around a true value of 0).
"""

import sys

sys.path.insert(0, '/opt/trn_rl_repo')

import numpy as np

_compiled = None


N_TOTAL = 8192
ROWS_PER_CORE = 1024
M_BLOCKS = 8
HALF = 4096
PHASES = 16


def _build_raw(reps=1, bench=False, tiny_io=False):
    import concourse.bass as bass
    import concourse.mybir as mybir

    DT = mybir.dt.float32
    DTR = mybir.dt.float32r
    F = mybir.ActivationFunctionType
    OP = mybir.AluOpType

    nc = bass.Bass()

    if tiny_io:
        nc.declare_dram_parameter("tiny", [128, 4], DT, isOutput=False)
        eta = nc.dram_tensor("eta", [128, N_TOTAL], DTR)
        etb = nc.dram_tensor("etb", [128, N_TOTAL], DTR)
        la = nc.dram_tensor("la", [128, ROWS_PER_CORE], DTR)
        lb = nc.dram_tensor("lb", [128, ROWS_PER_CORE], DTR)
        ubc = nc.dram_tensor("ubc", [128, N_TOTAL], DT)
        svec = nc.dram_tensor("svec", [128, 8], DT)
        dists_o = nc.dram_tensor("dists_i", [ROWS_PER_CORE, N_TOTAL], DT)
        probs_o = nc.dram_tensor("probs_i", [ROWS_PER_CORE, N_TOTAL], DT)
        done_o = nc.declare_dram_parameter("done_o", [128, 4], DT, isOutput=True)
    else:
        eta = nc.declare_dram_parameter("eta", [128, N_TOTAL], DTR, isOutput=False)
        etb = nc.declare_dram_parameter("etb", [128, N_TOTAL], DTR, isOutput=False)
        la = nc.declare_dram_parameter("la", [128, ROWS_PER_CORE], DTR, isOutput=False)
        lb = nc.declare_dram_parameter("lb", [128, ROWS_PER_CORE], DTR, isOutput=False)
        ubc = nc.declare_dram_parameter("ubc", [128, N_TOTAL], DT, isOutput=False)
        svec = nc.declare_dram_parameter("svec", [128, 8], DT, isOutput=False)
        dists_o = nc.declare_dram_parameter(
            "dists_o", [ROWS_PER_CORE, N_TOTAL], DT, isOutput=True)
        probs_o = nc.declare_dram_parameter(
            "probs_o", [ROWS_PER_CORE, N_TOTAL], DT, isOutput=True)
        done_o = None

    NIN = 6 * 16
    TOT = PHASES * reps

    def phase_mcol(p):
        q = p % PHASES
        return q // 2, (q % 2) * HALF

    from contextlib import ExitStack
    with ExitStack() as ctx:
        block = ctx.enter_context(nc.Block())
        dma_in = ctx.enter_context(nc.semaphore("dma_in"))
        pe_s = ctx.enter_context(nc.semaphore("pe_s"))
        dr_s = ctx.enter_context(nc.semaphore("dr_s"))
        ln_s = ctx.enter_context(nc.semaphore("ln_s"))
        h_s = ctx.enter_context(nc.semaphore("h_s"))
        z_s = ctx.enter_context(nc.semaphore("z_s"))
        o_s = ctx.enter_context(nc.semaphore("o_s"))
        dma_o = ctx.enter_context(nc.semaphore("dma_o"))
        t_eta = ctx.enter_context(nc.sbuf_tensor("t_eta", [128, N_TOTAL], DTR))
        t_etb = ctx.enter_context(nc.sbuf_tensor("t_etb", [128, N_TOTAL], DTR))
        t_la = ctx.enter_context(nc.sbuf_tensor("t_la", [128, ROWS_PER_CORE], DTR))
        t_lb = ctx.enter_context(nc.sbuf_tensor("t_lb", [128, ROWS_PER_CORE], DTR))
        t_ubc = ctx.enter_context(nc.sbuf_tensor("t_ubc", [128, N_TOTAL], DT))
        t_svec = ctx.enter_context(nc.sbuf_tensor("t_svec", [128, 8], DT))
        A0 = ctx.enter_context(nc.sbuf_tensor("A0", [128, HALF], DT))
        A1 = ctx.enter_context(nc.sbuf_tensor("A1", [128, HALF], DT))
        B0 = ctx.enter_context(nc.sbuf_tensor("B0", [128, HALF], DT))
        B1 = ctx.enter_context(nc.sbuf_tensor("B1", [128, HALF], DT))
        C0 = ctx.enter_context(nc.sbuf_tensor("C0", [128, HALF], DT))
        C1 = ctx.enter_context(nc.sbuf_tensor("C1", [128, HALF], DT))
        ps = ctx.enter_context(nc.psum_tensor("ps", [128, HALF], DT))

        A = [A0, A1]
        B = [B0, B1]
        C = [C0, C1]

        @block.sync
        def _(sync):
            for t, src in [(t_eta, eta), (t_etb, etb), (t_la, la),
                           (t_lb, lb), (t_ubc, ubc), (t_svec, svec)]:
                sync.dma_start(out=t[:], in_=src[:]).then_inc(dma_in, 16)
            for p in range(TOT):
                m, c0 = phase_mcol(p)
                s = p % 2
                sync.wait_ge(o_s, p + 1)
                sync.dma_start(
                    out=dists_o[m * 128:(m + 1) * 128, c0:c0 + HALF],
                    in_=C[s][:]).then_inc(dma_o, 16)
                sync.dma_start(
                    out=probs_o[m * 128:(m + 1) * 128, c0:c0 + HALF],
                    in_=A[s][:]).then_inc(dma_o, 16)
            sync.wait_ge(dma_o, 32 * TOT)

        @block.tensor
        def _(te):
            te.wait_ge(dma_in, NIN)
            for p in range(TOT):
                m, c0 = phase_mcol(p)
                if p >= 1:
                    te.wait_ge(dr_s, p)
                wla = t_la[:, m * 128:(m + 1) * 128]
                wlb = t_lb[:, m * 128:(m + 1) * 128]
                for sub in range(HALF // 512):
                    n0 = c0 + sub * 512
                    psl = ps[:, sub * 512:(sub + 1) * 512]
                    te.matmul(psl, wla, t_eta[:, n0:n0 + 512],
                              start=True, stop=False)
                    mm = te.matmul(psl, wlb, t_etb[:, n0:n0 + 512],
                                   start=False, stop=True)
                mm.then_inc(pe_s, 1)

        @block.vector
        def _(v):
            v.wait_ge(dma_in, NIN)
            for p in range(TOT + 1):
                s = p % 2
                if p < TOT:
                    m, c0 = phase_mcol(p)
                    if p >= 2:
                        v.wait_ge(dma_o, 32 * (p - 1))
                    v.wait_ge(pe_s, p + 1)
                    v.scalar_tensor_tensor(
                        out=A[s][:], in0=t_ubc[:, c0:c0 + HALF], scalar=0.0,
                        in1=ps[:], op0=OP.add, op1=OP.add)
                    v.scalar_tensor_tensor(
                        out=B[s][:], in0=t_ubc[:, c0:c0 + HALF],
                        scalar=t_svec[:, m:m + 1],
                        in1=ps[:], op0=OP.mult, op1=OP.add).then_inc(dr_s, 1)
                if p >= 1:
                    q = p - 1
                    sq = q % 2
                    v.wait_ge(ln_s, q + 1)
                    v.tensor_sub(out=A[sq][:], in0=A[sq][:],
                                 in1=B[sq][:]).then_inc(h_s, 1)

        @block.gpsimd
        def _(gp):
            for q in range(TOT):
                sq = q % 2
                gp.wait_ge(z_s, q + 1)
                gp.tensor_sub(out=C[sq][:], in0=C[sq][:], in1=A[sq][:])
                gp.tensor_scalar(
                    out=A[sq][:], in0=B[sq][:], scalar1=-0.5, scalar2=0.5,
                    op0=OP.mult, op1=OP.add).then_inc(o_s, 1)
            if bench:
                gp.wait_ge(o_s, TOT)
                gp.memset(A[0][:, 0:4], 0.0)
                gp.dma_start(out=done_o[:],
                             in_=A[0][:, 0:4]).then_inc(dma_o, 16)

        @block.scalar
        def _(sc):
            sc.wait_ge(dma_in, NIN)
            for p in range(TOT + 1):
                s = p % 2
                if p < TOT:
                    m, c0 = phase_mcol(p)
                    sc.wait_ge(dr_s, p + 1)
                    sc.activation(A[s][:], A[s][:], F.Ln,
                                  bias=t_svec[:, m:m + 1], scale=1.0)
                    sc.activation(B[s][:], B[s][:], F.Ln,
                                  bias=1.0, scale=1.0).then_inc(ln_s, 1)
                if p >= 1:
                    q = p - 1
                    sq = q % 2
                    sc.wait_ge(h_s, q + 1)
                    if q >= 2:
                        sc.wait_ge(dma_o, 32 * (q - 1))
                    sc.activation(B[sq][:], A[sq][:], F.Exp,
                                  bias=0.0, scale=0.5)
                    sc.activation(C[sq][:], B[sq][:], F.Ln,
                                  bias=1.0, scale=1.0)
                    sc.activation(A[sq][:], B[sq][:], F.Ln,
                                  bias=1.0, scale=-1.0).then_inc(z_s, 1)


    return nc


def _prepare_in_maps_raw(embeddings):
    E = np.ascontiguousarray(embeddings, dtype=np.float32)
    x2 = (E.astype(np.float64) ** 2).sum(axis=1)
    ET = E.T
    ETn2 = (-2.0 * ET).astype(np.float32)
    x2f = x2.astype(np.float32)

    eta = np.ascontiguousarray(ET[:128])
    etb = np.ascontiguousarray(ET[128:])
    ubc = np.ascontiguousarray(np.broadcast_to(x2f[None, :], (128, N_TOTAL)))

    in_maps = []
    for c in range(8):
        rs = slice(c * ROWS_PER_CORE, (c + 1) * ROWS_PER_CORE)
        sv = np.ascontiguousarray(x2f[rs].reshape(8, 128).T)  # svec[p, m]
        in_maps.append({
            "eta": eta, "etb": etb,
            "la": np.ascontiguousarray(ETn2[:128, rs]),
            "lb": np.ascontiguousarray(ETn2[128:, rs]),
            "ubc": ubc, "svec": sv,
        })
    return in_maps


def kernel(embeddings: np.ndarray) -> tuple[np.ndarray, np.ndarray]:
    global _compiled
    from concourse.bass_utils import run_bass_kernel_spmd

    if _compiled is None:
        _compiled = _build_raw()
    nc = _compiled

    in_maps = _prepare_in_maps_raw(embeddings)
    res = run_bass_kernel_spmd(nc, in_maps, list(range(8)))

    dists = np.empty((N_TOTAL, N_TOTAL), np.float32)
    probs = np.empty((N_TOTAL, N_TOTAL), np.float32)
    for c in range(8):
        rs = slice(c * ROWS_PER_CORE, (c + 1) * ROWS_PER_CORE)
        dists[rs] = res.results[c]["dists_o"]
        probs[rs] = res.results[c]["probs_o"]

    idx = np.arange(N_TOTAL)
    dists[idx, idx] = 0.0
    probs[idx, idx] = 0.0
    return (probs, dists)



# revision 2
# speedup vs baseline: 14.4312x; 14.4312x over previous
"""Poincare pairwise edge generator on 8 Trainium2 NeuronCores.

Math: for the Poincare-ball distance with c=1 the mobius-norm numerator
factors exactly: num2 = A^2|x|^2 - 2AB<x,y> + B^2|y|^2 = sqdist * D with
sqdist = |x-y|^2 and D = 1 - 2<x,y> + |x|^2|y|^2, so
  z = sqrt(sqdist/D) = exp(0.5*(ln(20*sqdist) - ln D) - 0.5*ln 20)
  dists = ln(1+z) - ln(1-z)        (= 2 artanh z)
  probs = -0.5*z + 0.5             (= sigmoid(-dists), exact)
The *20 rescale centers ln(sqdist) near 0 so bf16 storage of the log
intermediates costs ~1e-3 instead of ~1e-2 absolute error.

Structure: this axon-tunneled stack charges ~40us per *static* engine
instruction per NEFF execution but only ~0.5-1us per instruction executed
inside a Fori hardware loop, so the whole 16-phase pipeline lives in
per-engine Fori loops with register-computed AP offsets (~130 static
instructions total, independent of reps). Per phase p (m=(p%16)>>1 row
block, col0=(p&1)*4096):
  PE:   16 matmuls (8 PSUM banks x K=256 as 2 bf16 matmuls, moving 512)
        -> ps = -2<x,y>; weights come from a staging slot because walrus
        forbids register offsets in ldweights (DVE restages per phase)
  DVE:  stt A[s]=(u+s_i)+ps=sqdist, B[s]=(u*s_i)+ps=D-1 (bf16), then
        skewed h(p-1) = lnA - lnB -> D4 (f32)
  ACT:  Ln(20*A)->A, Ln(B+1)->B; skewed Exp(.5h-.5ln20)->D4 (z, f32),
        Ln(1+z)->T, Ln(1-z)->A, Copy(-0.5z+0.5)->B (probs)
        (Ln/Exp share one act table set -> zero table reloads; the Copy
        on ACT because Pool tensor_scalar mishandles in-place/f32->bf16)
  Pool: dists = T - A -> D4 (f32)
  SYNC: dma D4 -> dists_o (f32), B -> probs_o (bf16); 4-way If_eq picks
        the static SBUF source (DMA cannot take symbolic SBUF APs)
4 bf16 buffer sets (s=p&3) + 2 f32 D4 sets decouple the pipeline stages.

Sharding: rows across 8 cores (1024 each); every core holds the full
[256, 8192] transposed embeddings. Diagonal fixed on host at unshard
(probs/dists diagonals are exactly 0; the on-device values there are
fp cancellation noise / NaN from ln(<=0) and are discarded).
"""

import sys

sys.path.insert(0, '/opt/trn_rl_repo')

import numpy as np

_compiled = None

N_TOTAL = 8192
ROWS_PER_CORE = 1024
HALF = 4096
PHASES = 16
LN20 = 2.995732273553991


def _build_raw(reps=1, bench=False, tiny_io=False):
    import concourse.bass as bass
    import concourse.mybir as mybir

    DT = mybir.dt.float32
    BF = mybir.dt.bfloat16
    F = mybir.ActivationFunctionType
    OP = mybir.AluOpType

    nc = bass.Bass()

    if tiny_io:
        nc.declare_dram_parameter("tiny", [128, 4], DT, isOutput=False)
        emb2 = nc.dram_tensor("emb2", [128, 2 * N_TOTAL], BF)
        lab2 = nc.dram_tensor("lab2", [128, 2 * ROWS_PER_CORE], BF)
        ubc = nc.dram_tensor("ubc", [128, N_TOTAL], DT)
        sv = nc.dram_tensor("sv", [128, 8], DT)
        dists_o = nc.dram_tensor("dists_i", [ROWS_PER_CORE, N_TOTAL], DT)
        probs_o = nc.dram_tensor("probs_i", [ROWS_PER_CORE, N_TOTAL], BF)
        done_o = nc.declare_dram_parameter("done_o", [128, 4], DT, isOutput=True)
    else:
        emb2 = nc.declare_dram_parameter("emb2", [128, 2 * N_TOTAL], BF,
                                         isOutput=False)
        lab2 = nc.declare_dram_parameter("lab2", [128, 2 * ROWS_PER_CORE], BF,
                                         isOutput=False)
        ubc = nc.declare_dram_parameter("ubc", [128, N_TOTAL], DT,
                                        isOutput=False)
        sv = nc.declare_dram_parameter("sv", [128, 8], DT, isOutput=False)
        dists_o = nc.declare_dram_parameter(
            "dists_o", [ROWS_PER_CORE, N_TOTAL], DT, isOutput=True)
        probs_o = nc.declare_dram_parameter(
            "probs_o", [ROWS_PER_CORE, N_TOTAL], BF, isOutput=True)
        done_o = None

    TOT = PHASES * reps

    # register the Exp bias constant (-ln(20)/2) as a const AP for activation
    EB = -0.5 * LN20
    cb = nc.alloc_sbuf_tensor("cexp", [128, 1], DT)
    nc.gpsimd.memset(cb.ap(), EB)
    nc.const_aps.aps[(DT, EB)] = cb.ap()
    nc.all_engine_barrier()

    from contextlib import ExitStack
    with ExitStack() as ctx:
        block = ctx.enter_context(nc.Block())
        dma_in = ctx.enter_context(nc.semaphore("dma_in"))
        w_s = ctx.enter_context(nc.semaphore("w_s"))
        pe_s = ctx.enter_context(nc.semaphore("pe_s"))
        dr_s = ctx.enter_context(nc.semaphore("dr_s"))
        ln_s = ctx.enter_context(nc.semaphore("ln_s"))
        h_s = ctx.enter_context(nc.semaphore("h_s"))
        z_s = ctx.enter_context(nc.semaphore("z_s"))
        o_s = ctx.enter_context(nc.semaphore("o_s"))
        dma_o = ctx.enter_context(nc.semaphore("dma_o"))

        t_eta = ctx.enter_context(nc.sbuf_tensor("t_eta", [128, N_TOTAL], BF))
        t_etb = ctx.enter_context(nc.sbuf_tensor("t_etb", [128, N_TOTAL], BF))
        t_lab = ctx.enter_context(
            nc.sbuf_tensor("t_lab", [128, 2 * ROWS_PER_CORE], BF))
        t_ubc = ctx.enter_context(nc.sbuf_tensor("t_ubc", [128, N_TOTAL], DT))
        t_sv = ctx.enter_context(nc.sbuf_tensor("t_sv", [128, 8], DT))
        stag = ctx.enter_context(nc.sbuf_tensor("stag", [128, 2 * 256], BF))
        A = ctx.enter_context(nc.sbuf_tensor("A", [128, 4 * HALF], BF))
        B = ctx.enter_context(nc.sbuf_tensor("B", [128, 4 * HALF], BF))
        T = ctx.enter_context(nc.sbuf_tensor("T", [128, 4 * HALF], BF))
        D4 = ctx.enter_context(nc.sbuf_tensor("D4", [128, 2 * HALF], DT))
        ps = ctx.enter_context(nc.psum_tensor("ps", [128, HALF], DT))

        L2 = 2 * ROWS_PER_CORE
        W4 = 4 * HALF

        def uslice(off):        # ubc[:, col0:col0+HALF]
            return bass.AP(t_ubc, off, [[N_TOTAL, 128], [1, HALF]])

        def svap(m):            # sv[:, m:m+1]
            return bass.AP(t_sv, m, [[8, 128], [1, 1]])

        def wslice(t, off):     # 4-set work buf [:, off:off+HALF]
            return bass.AP(t, off, [[W4, 128], [1, HALF]])

        def d4slice(off):       # D4 (2 sets) [:, off:off+HALF]
            return bass.AP(D4, off, [[2 * HALF, 128], [1, HALF]])

        def easlice(off):       # eta [:, off:off+512] moving operand
            return bass.AP(t_eta, off, [[N_TOTAL, 128], [1, 512]])

        def ebslice(off):       # etb [:, off:off+512] moving operand
            return bass.AP(t_etb, off, [[N_TOTAL, 128], [1, 512]])

        def lslice(off):        # lab [:, off:off+128] weight-copy source
            return bass.AP(t_lab, off, [[L2, 128], [1, 128]])

        def oslice(t, off):     # DRAM out [m*128:(m+1)*128, col0:col0+HALF]
            return bass.AP(t, off, [[N_TOTAL, 128], [1, HALF]])

        @block.sync
        def _(sync):
            sync.dma_start(out=t_eta[:],
                           in_=emb2[:, 0:N_TOTAL]).then_inc(dma_in, 16)
            sync.dma_start(out=t_etb[:],
                           in_=emb2[:, N_TOTAL:2 * N_TOTAL]).then_inc(dma_in, 16)
            for t, src in [(t_lab, lab2), (t_ubc, ubc), (t_sv, sv)]:
                sync.dma_start(out=t[:], in_=src[:]).then_inc(dma_in, 16)
            sync.sem_inc(dma_o, 96)  # bias so buffer-reuse waits never go < 0
            if TOT == 0:
                return
            with sync.Fori(0, TOT) as p:
                d_off = sync.compute_val(
                    (((p & 15) >> 1) << 20) + ((p & 1) << 12))
                sidx = sync.compute_val(p & 3)
                sync.wait_ge(o_s, sync.compute_val(p + 1))
                for k in range(4):
                    with sync.If_eq(sidx, k):
                        sync.dma_start(
                            out=oslice(dists_o, d_off),
                            in_=D4[:, (k & 1) * HALF:((k & 1) + 1) * HALF]
                        ).then_inc(dma_o, 16)
                        sync.dma_start(
                            out=oslice(probs_o, d_off),
                            in_=B[:, k * HALF:(k + 1) * HALF]
                        ).then_inc(dma_o, 16)
            sync.wait_ge(dma_o, 32 * TOT + 96)

        @block.tensor
        def _(te):
            te.wait_ge(dma_in, 64)
            if TOT == 0:
                return
            with te.Fori(0, TOT) as p:
                col0 = te.compute_val((p & 1) << 12)
                te.wait_ge(w_s, te.compute_val(p + 1))
                te.wait_ge(dr_s, p)
                for b in range(8):
                    psl = ps[:, b * 512:(b + 1) * 512]
                    ea = te.compute_val(col0 + b * 512)
                    te.matmul(psl, stag[:, 0:128], easlice(ea),
                              start=True, stop=False)
                    mm = te.matmul(psl, stag[:, 128:256], ebslice(ea),
                                   start=False, stop=True)
                    if b == 7:
                        mm.then_inc(pe_s, 1)

        @block.vector
        def _(v):
            v.wait_ge(dma_in, 80)
            # stage weights for phase 0 (m=0)
            v.tensor_copy(out=stag[:, 0:128], in_=t_lab[:, 0:128])
            v.tensor_copy(out=stag[:, 128:256],
                          in_=t_lab[:, ROWS_PER_CORE:ROWS_PER_CORE + 128])
            v.sem_inc(w_s, 1)
            if TOT == 0:
                return
            with v.Fori(0, TOT + 1) as p:
                with v.If_lt(p, TOT):
                    m = v.compute_val((p & 15) >> 1)
                    col0 = v.compute_val((p & 1) << 12)
                    s_off = v.compute_val((p & 3) << 12)
                    v.wait_ge(dma_o, v.compute_val(p << 5))
                    v.wait_ge(pe_s, v.compute_val(p + 1))
                    # stage weights for phase p+1 (PE(p) is done with stag)
                    m1 = v.compute_val((((p + 1) & 15) >> 1) << 7)
                    m1b = v.compute_val(((((p + 1) & 15) >> 1) << 7)
                                        + ROWS_PER_CORE)
                    v.tensor_copy(out=stag[:, 0:128], in_=lslice(m1))
                    v.tensor_copy(out=stag[:, 128:256], in_=lslice(m1b))
                    v.sem_inc(w_s, 1)
                    v.scalar_tensor_tensor(
                        out=wslice(A, s_off), in0=uslice(col0), scalar=svap(m),
                        in1=ps[:], op0=OP.add, op1=OP.add)
                    v.scalar_tensor_tensor(
                        out=wslice(B, s_off), in0=uslice(col0), scalar=svap(m),
                        in1=ps[:], op0=OP.mult, op1=OP.add).then_inc(dr_s, 1)
                with v.If_ne(p, 0):
                    q_off = v.compute_val(((p - 1) & 3) << 12)
                    q2_off = v.compute_val(((p - 1) & 1) << 12)
                    v.wait_ge(dma_o, v.compute_val((p << 5) + 32))
                    v.wait_ge(ln_s, p)
                    v.tensor_sub(out=d4slice(q2_off), in0=wslice(A, q_off),
                                 in1=wslice(B, q_off)).then_inc(h_s, 1)

        @block.scalar
        def _(sc):
            if TOT == 0:
                return
            with sc.Fori(0, TOT + 1) as p:
                with sc.If_lt(p, TOT):
                    s_off = sc.compute_val((p & 3) << 12)
                    sc.wait_ge(dr_s, sc.compute_val(p + 1))
                    sc.activation(wslice(A, s_off), wslice(A, s_off), F.Ln,
                                  bias=0.0, scale=20.0)
                    sc.activation(wslice(B, s_off), wslice(B, s_off), F.Ln,
                                  bias=1.0, scale=1.0).then_inc(ln_s, 1)
                with sc.If_ne(p, 0):
                    q_off = sc.compute_val(((p - 1) & 3) << 12)
                    q2_off = sc.compute_val(((p - 1) & 1) << 12)
                    sc.wait_ge(h_s, p)
                    sc.activation(d4slice(q2_off), d4slice(q2_off), F.Exp,
                                  bias=-0.5 * LN20, scale=0.5)
                    sc.activation(wslice(T, q_off), d4slice(q2_off), F.Ln,
                                  bias=1.0, scale=1.0)
                    sc.activation(wslice(A, q_off), d4slice(q2_off), F.Ln,
                                  bias=1.0, scale=-1.0)
                    sc.activation(wslice(B, q_off), d4slice(q2_off), F.Copy,
                                  bias=0.5, scale=-0.5).then_inc(z_s, 1)

        @block.gpsimd
        def _(gp):
            if TOT == 0:
                if bench:
                    gp.memset(A[:, 0:4], 0.0)
                    gp.dma_start(out=done_o[:],
                                 in_=A[:, 0:4]).then_inc(dma_o, 16)
                return
            with gp.Fori(0, TOT) as q:
                q_off = gp.compute_val((q & 3) << 12)
                q2_off = gp.compute_val((q & 1) << 12)
                gp.wait_ge(z_s, gp.compute_val(q + 1))
                gp.tensor_sub(out=d4slice(q2_off), in0=wslice(T, q_off),
                              in1=wslice(A, q_off)).then_inc(o_s, 1)
            if bench:
                gp.wait_ge(o_s, TOT)
                gp.memset(A[:, 0:4], 0.0)
                gp.dma_start(out=done_o[:],
                             in_=A[:, 0:4]).then_inc(dma_o, 16)

    return nc


def _prepare_in_maps_raw(embeddings):
    import ml_dtypes
    E = np.ascontiguousarray(embeddings, dtype=np.float32)
    x2 = (E.astype(np.float64) ** 2).sum(axis=1).astype(np.float32)
    ET = E.T  # [256, 8192]
    emb2 = np.ascontiguousarray(
        np.concatenate([ET[:128], ET[128:]], axis=1)).astype(ml_dtypes.bfloat16)
    ETn2 = (-2.0 * ET).astype(ml_dtypes.bfloat16)
    ubc = np.ascontiguousarray(np.broadcast_to(x2[None, :], (128, N_TOTAL)))

    in_maps = []
    for c in range(8):
        rs = slice(c * ROWS_PER_CORE, (c + 1) * ROWS_PER_CORE)
        lab2 = np.ascontiguousarray(
            np.concatenate([ETn2[:128, rs], ETn2[128:, rs]], axis=1))
        sv = np.ascontiguousarray(x2[rs].reshape(8, 128).T)  # sv[p, m]
        in_maps.append({"emb2": emb2, "lab2": lab2, "ubc": ubc, "sv": sv})
    return in_maps


def kernel(embeddings: np.ndarray) -> tuple[np.ndarray, np.ndarray]:
    global _compiled
    from concourse.bass_utils import run_bass_kernel_spmd

    if _compiled is None:
        _compiled = _build_raw()
    nc = _compiled

    in_maps = _prepare_in_maps_raw(embeddings)
    res = run_bass_kernel_spmd(nc, in_maps, list(range(8)))

    dists = np.empty((N_TOTAL, N_TOTAL), np.float32)
    probs = np.empty((N_TOTAL, N_TOTAL), np.float32)
    for c in range(8):
        rs = slice(c * ROWS_PER_CORE, (c + 1) * ROWS_PER_CORE)
        dists[rs] = res.results[c]["dists_o"]
        probs[rs] = res.results[c]["probs_o"].astype(np.float32)

    idx = np.arange(N_TOTAL)
    dists[idx, idx] = 0.0
    probs[idx, idx] = 0.0
    return (probs, dists)
